# revision 1
# baseline (speedup 1.0000x reference)
"""Trainium2 Bass kernel for nn_DenseRNNBase (GRU with dense skip history).

Sharding: data-parallel over batch B=128 across 8 cores. Each core covers 32
batch rows (its 16 primary + the next core's 16, wrapped — a double cover so
the PSUM regions and matmul M-dims are 32-aligned) and writes out only its 16
primary rows. Zero inter-core communication.

Per-core recurrence (step t, k=t%8, j=t%8 block-local):
  gates(t) = h[t-1] @ (W_hh [+W_dense[0] if k>=1]) + sum_{d=1}^{k-1} h[t-1-d] @ W_dense[d]
             + x_t @ W_ih + biases
  r,z = sigmoid(...)   n = tanh(i_n + r*(h_n + b_hh_n))   h[t] = z*h[t-1] + (1-z)*n

Key structure:
- All of a step's gate pre-activations accumulate in one PSUM "region":
  a [32, 1536] slice of a single [128, 1536] PSUM area at partition offset
  32*(j%4). Writers: dense-lag PACK matmuls (one weight-stream covers several
  future steps via the stationary M dim), the W_hh matmul, an identity-matmul
  inject of the x-projection (r,z slices), and a K=1 ones-row matmul adding
  b_hh_n to the n slice.
- Within a block of 8 steps every dense-skip source lies inside the block
  (t%8 resets), so an 8-slot ring of transposed hidden states (refreshed by 4
  PE transposes per step) feeds all matmuls.
- Weight matrices stream as the moving operand (bf16, 1 col/cycle); the
  x-projection runs in float32r (fp32 bits at full PE rate).
- The GRU cell runs on DVE/ACT at the region's partition offset (engine ops
  require matching operand base partitions).
"""

import sys

for _p in ("/opt/trn_rl_repo",):
    if _p not in sys.path:
        sys.path.insert(0, _p)

import numpy as np
import ml_dtypes

import concourse.bass as bass
import concourse.bacc as bacc
import concourse.tile as tile
import concourse.mybir as mybir
from concourse import bass_utils

F32 = mybir.dt.float32
F32R = mybir.dt.float32r
BF16 = mybir.dt.bfloat16
AF = mybir.ActivationFunctionType

T, B, I, H, D = 256, 128, 512, 512, 8
G = 3 * H  # 1536
NCORES = 8
BL = 32   # local batch (double cover)
BP = 16   # primary batch rows written out
KI = I // 128
KH = H // 128
NG = G // 512


def build_nc(t_steps=T, reps=1):
    nc = bacc.Bacc("TRN2", target_bir_lowering=False, debug=False)

    xT_d = nc.dram_tensor("xT", (I, t_steps * BL), F32R, kind="ExternalInput")
    h0T_d = nc.dram_tensor("h0T", (H, BL), F32, kind="ExternalInput")
    h0n_d = nc.dram_tensor("h0n", (BL, H), F32, kind="ExternalInput")
    wih_d = nc.dram_tensor("wih", (I, G), F32R, kind="ExternalInput")
    whh_d = nc.dram_tensor("whh", (H, G), BF16, kind="ExternalInput")
    whh0_d = nc.dram_tensor("whh0", (H, G), BF16, kind="ExternalInput")
    wd_d = [nc.dram_tensor(f"wd{d}", (H, G), BF16, kind="ExternalInput")
            for d in range(1, 7)]
    bias_d = nc.dram_tensor("biasb", (128, G), F32, kind="ExternalInput")
    identb_d = nc.dram_tensor("identb", (128, BL), BF16, kind="ExternalInput")
    identf_d = nc.dram_tensor("identf", (128, BL), F32, kind="ExternalInput")
    bhhn_d = nc.dram_tensor("bhhn", (1, H), BF16, kind="ExternalInput")
    ones_d = nc.dram_tensor("ones", (1, BL), BF16, kind="ExternalInput")
    ys_d = nc.dram_tensor("ys", (t_steps, BP, H), F32, kind="ExternalOutput")

    nblk = t_steps // 8

    with tile.TileContext(nc) as tc:
        with (
            tc.tile_pool(name="wpool", bufs=1) as wpool,
            tc.tile_pool(name="cpool", bufs=1) as cpool,
            tc.tile_pool(name="xt", bufs=2) as xtp,
            tc.tile_pool(name="sbel", bufs=2) as sbp,
            tc.tile_pool(name="hyp", bufs=2) as hyp,
            tc.tile_pool(name="ct", bufs=1) as ct,
            tc.tile_pool(name="pa", bufs=1, space="PSUM") as pap,
            tc.tile_pool(name="px", bufs=2, space="PSUM") as pxp,
            tc.tile_pool(name="pt", bufs=2, space="PSUM") as ptp,
        ):
            # ---- resident weights ----
            wih = wpool.tile([128, KI, G], F32R, tag="wih")
            whh = wpool.tile([128, KH, G], BF16, tag="whh")
            whh0 = wpool.tile([128, KH, G], BF16, tag="whh0")
            wd = [wpool.tile([128, KH, G], BF16, tag=f"wd{d}", name=f"wd{d}")
                  for d in range(6)]
            for k in range(KI):
                nc.sync.dma_start(wih[:, k, :], wih_d[k * 128:(k + 1) * 128, :])
            for k in range(KH):
                nc.sync.dma_start(whh[:, k, :], whh_d[k * 128:(k + 1) * 128, :])
                nc.sync.dma_start(whh0[:, k, :], whh0_d[k * 128:(k + 1) * 128, :])
                for d in range(6):
                    nc.sync.dma_start(wd[d][:, k, :], wd_d[d][k * 128:(k + 1) * 128, :])

            bias_s = cpool.tile([128, G], F32, tag="bias")
            nc.sync.dma_start(bias_s[:], bias_d[:])
            # 32x32 identity replicated on each 32-partition group
            ident_s = cpool.tile([128, BL], BF16, tag="ident")
            nc.sync.dma_start(ident_s[:], identb_d[:])
            identf_s = cpool.tile([128, BL], F32, tag="identf")
            nc.sync.dma_start(identf_s[:], identf_d[:])
            bhhn_f = cpool.tile([1, H], BF16, tag="bhhnf")
            nc.sync.dma_start(bhhn_f[:], bhhn_d[:])
            ones_f = cpool.tile([1, BL], BF16, tag="onesf")
            nc.sync.dma_start(ones_f[:], ones_d[:])

            # transposed hidden-state ring: slot s holds hyT of step t%8==s
            hist = cpool.tile([128, KH, 8, BL], BF16, tag="hist")
            h0T_s = cpool.tile([128, KH, BL], F32, tag="h0Ts")
            for k in range(KH):
                nc.sync.dma_start(h0T_s[:, k, :], h0T_d[k * 128:(k + 1) * 128, :])

            for rep in range(reps):
                for k in range(KH):
                    nc.vector.tensor_copy(hist[:, k, 7, :], h0T_s[:, k, :])
                h_prev = hyp.tile([128, H], F32, tag="hy", name=f"hprev{rep}")
                nc.sync.dma_start(h_prev[96:128, :], h0n_d[:])
                po_prev = 96

                def xproj_load(m):
                    """DMA the x slice for block m and allocate its s_bel."""
                    xt_t = xtp.tile([128, KI, 8 * BL], F32R, tag="xt",
                                    name=f"xt_{rep}_{m}")
                    for k in range(KI):
                        nc.sync.dma_start(
                            xt_t[:, k, :],
                            xT_d[k * 128:(k + 1) * 128, m * 8 * BL:(m + 1) * 8 * BL])
                    sb = sbp.tile([128, 2, 2 * H], BF16, tag="sb",
                                  name=f"sb_{rep}_{m}")
                    sbn = sbp.tile([128, 2, H], F32, tag="sbn",
                                   name=f"sbn_{rep}_{m}")
                    return xt_t, sb, sbn

                def xproj_piece(blk, piece):
                    """One (g, n) x-projection matmul group + eviction."""
                    xt_t, sb, sbn = blk
                    g, n = divmod(piece, NG)
                    px_t = pxp.tile([128, 512], F32, tag="px")
                    for k in range(KI):
                        nc.tensor.matmul(
                            px_t[:],
                            xt_t[:, k, g * 128:(g + 1) * 128],
                            wih[:, k, n * 512:(n + 1) * 512],
                            start=(k == 0), stop=(k == KI - 1))
                    dst = (sb[:, g, n * 512:(n + 1) * 512] if n < 2
                           else sbn[:, g, :])
                    nc.vector.tensor_add(
                        dst, px_t[:], bias_s[:, n * 512:(n + 1) * 512])

                blk_cur = xproj_load(0)
                for p in range(2 * NG):
                    xproj_piece(blk_cur, p)
                blk_next = None

                for m in range(nblk):
                    _, sb, sbn = blk_cur
                    area = pap.tile([128, G], F32, tag="area")

                    def single(jt, d):
                        """One unpacked lag-d term into step jt's region."""
                        pb = 32 * (jt % 4)
                        tps = (0, 96) if pb == 96 else None
                        for k in range(KH):
                            for n in range(NG):
                                nc.tensor.matmul(
                                    area[pb:pb + 32, n * 512:(n + 1) * 512],
                                    hist[:, k, jt - 1 - d, :],
                                    wd[d - 1][:, k, n * 512:(n + 1) * 512],
                                    start=False, stop=False,
                                    tile_position=tps,
                                    skip_group_check=True)

                    def pack(d, j1, cnt, start):
                        """Dense-lag pack: one W_dense[d] weight stream fills
                        regions j1..j1+cnt-1 via stationary-M packing."""
                        s0 = j1 - 1 - d
                        pb = 32 * (j1 % 4)
                        for k in range(KH):
                            for n in range(NG):
                                nc.tensor.matmul(
                                    area[pb:pb + 32 * cnt, n * 512:(n + 1) * 512],
                                    hist[:, k, s0:s0 + cnt, :],
                                    wd[d - 1][:, k, n * 512:(n + 1) * 512],
                                    start=(start and k == 0), stop=False,
                                    skip_group_check=True)

                    for j in range(8):
                        t = 8 * m + j
                        po = 32 * (j % 4)
                        gi = j // 4
                        tp96 = (0, 96) if po == 96 else None

                        # NOTE: region(j) shares its PSUM slot with region
                        # (j+4), so writers for steps 4..7 may only be
                        # emitted once the slot's previous reader (chain of
                        # step j-4) has been emitted.
                        if j == 2:
                            pack(1, 2, 2, start=True)   # starts regions 2,3
                        elif j == 4:
                            pack(3, 4, 4, start=True)   # starts regions 4..7
                            pack(1, 4, 2, start=False)
                            pack(2, 4, 2, start=False)
                            single(3 + 4, 6)            # lag6 -> step 7
                        elif j == 5:
                            pack(4, 5, 1, start=False)
                            pack(4, 6, 2, start=False)
                            pack(5, 6, 2, start=False)
                        elif j == 6:
                            pack(1, 6, 2, start=False)
                            pack(2, 6, 2, start=False)

                        # W_hh (or W_hh + W_dense[0]) term
                        wsel = whh0 if j >= 1 else whh
                        for k in range(KH):
                            for n in range(NG):
                                nc.tensor.matmul(
                                    area[po:po + 32, n * 512:(n + 1) * 512],
                                    hist[:, k, (t - 1) % 8, :],
                                    wsel[:, k, n * 512:(n + 1) * 512],
                                    start=(j <= 1 and k == 0), stop=False,
                                    tile_position=tp96, skip_group_check=True)
                        if j == 3:
                            single(3, 2)                # lag2 -> step 3
                        # inject x-projection r,z slices (with biases)
                        tpii = (96, 96) if po == 96 else None
                        for n in range(2):
                            nc.tensor.matmul(
                                area[po:po + 32, n * 512:(n + 1) * 512],
                                ident_s[po:po + BL, :],
                                sb[po:po + BL, gi, n * 512:(n + 1) * 512],
                                start=False, stop=True, tile_position=tpii,
                                skip_group_check=True)
                        # n slice += b_hh_n (broadcast over batch rows)
                        nc.tensor.matmul(
                            area[po:po + 32, 1024:1536],
                            ones_f[:], bhhn_f[:],
                            start=False, stop=True, tile_position=tp96,
                            skip_group_check=True)

                        # ---- GRU cell at partition offset po ----
                        s = slice(po, po + 32)
                        h_cur = ct.tile([128, H], F32, tag="hc")
                        nc.gpsimd.tensor_copy(
                            h_cur[s, :], h_prev[po_prev:po_prev + 32, :])
                        i_n0 = ct.tile([128, H], F32, tag="in0")
                        nc.gpsimd.tensor_copy(i_n0[s, :], sbn[s, gi, :])
                        r = ct.tile([128, H], F32, tag="r")
                        nc.scalar.activation(r[s, :], area[s, 0:512], AF.Sigmoid)
                        z = ct.tile([128, H], F32, tag="z")
                        nc.scalar.activation(z[s, :], area[s, 512:1024], AF.Sigmoid)
                        w1 = ct.tile([128, H], F32, tag="w1")  # 1 - z
                        nc.gpsimd.tensor_scalar(
                            w1[s, :], z[s, :], -1.0, 1.0,
                            mybir.AluOpType.mult, mybir.AluOpType.add)
                        rn = ct.tile([128, H], F32, tag="rn")
                        a_n = ct.tile([128, H], F32, tag="an")
                        nn = ct.tile([128, H], F32, tag="nn")
                        zh = ct.tile([128, H], F32, tag="zh")
                        nc.gpsimd.tensor_mul(zh[s, :], z[s, :], h_cur[s, :])
                        wn = ct.tile([128, H], F32, tag="wn")
                        for half in range(2):
                            cs = slice(half * 256, (half + 1) * 256)
                            ps = slice(1024 + half * 256, 1024 + (half + 1) * 256)
                            nc.vector.tensor_mul(rn[s, cs], r[s, cs], area[s, ps])
                            nc.vector.tensor_add(a_n[s, cs], rn[s, cs], i_n0[s, cs])
                            nc.scalar.activation(nn[s, cs], a_n[s, cs], AF.Tanh)
                            nc.vector.tensor_mul(wn[s, cs], w1[s, cs], nn[s, cs])
                        # produce hy in two column halves so the transposed
                        # ring (and the next step's W_hh chunks) can start on
                        # the first half while the second is still computing
                        hy = hyp.tile([128, H], F32, tag="hy")
                        ptt = ptp.tile([128, KH, BL], F32, tag="pt")
                        for half in range(2):
                            cs = slice(half * 256, (half + 1) * 256)
                            nc.vector.tensor_add(hy[s, cs], zh[s, cs], wn[s, cs])
                            for k in (2 * half, 2 * half + 1):
                                nc.tensor.transpose(
                                    ptt[:, k, :], hy[s, k * 128:(k + 1) * 128],
                                    identf_s[s, :], tile_position=(po, 0))
                                nc.scalar.copy(hist[:, k, j, :], ptt[:, k, :])

                        nc.gpsimd.dma_start(ys_d[t], hy[po:po + BP, :])
                        # prefetch next block's x-projection, one piece per
                        # step, emitted last so it fills PE gaps while this
                        # step's cell runs
                        if m + 1 < nblk:
                            if j == 0:
                                blk_next = xproj_load(m + 1)
                            if j < 2 * NG:
                                xproj_piece(blk_next, j)
                        h_prev = hy
                        po_prev = po
                    blk_cur = blk_next

    nc.compile()
    return nc


def round_f32r(a):
    """Round fp32 to the PE's float32r (tf32-like 10-bit mantissa)."""
    a = np.ascontiguousarray(a, dtype=np.float32)
    v = a.view(np.uint32).copy()
    v += 0x1000 + ((v >> 13) & 1)
    v &= 0xFFFFE000
    return v.view(np.float32)


def host_prep(x, h0, W_ih, W_hh, b_ih, b_hh, W_dense, t_steps=T):
    """Build per-core in_maps."""
    bf = ml_dtypes.bfloat16
    whh0 = (W_hh + W_dense[0]).astype(bf)
    whh_b = W_hh.astype(bf)
    wd_b = [W_dense[d].astype(bf) for d in range(1, 7)]
    bias_row = np.concatenate([(b_ih + b_hh)[:2 * H], b_ih[2 * H:]])
    bias = np.broadcast_to(bias_row[None, :], (128, G)).astype(np.float32).copy()
    bhhn = b_hh[2 * H:].reshape(1, H).astype(bf)
    ones = np.ones((1, BL), bf)
    ident = np.tile(np.eye(BL, dtype=np.float32), (4, 1))
    wih_f = round_f32r(W_ih)

    in_maps = []
    for c in range(NCORES):
        idx = (16 * c + np.arange(BL)) % B
        xc = x[:t_steps, idx, :]
        xT = np.ascontiguousarray(xc.transpose(2, 0, 1).reshape(I, t_steps * BL))
        m = {
            "xT": round_f32r(xT),
            "h0T": np.ascontiguousarray(h0[idx].T).astype(np.float32),
            "h0n": h0[idx].astype(np.float32),
            "wih": wih_f, "whh": whh_b, "whh0": whh0,
            "biasb": bias, "identb": ident.astype(bf), "identf": ident,
            "bhhn": bhhn, "ones": ones,
        }
        for d in range(6):
            m[f"wd{d + 1}"] = wd_b[d]
        in_maps.append(m)
    return in_maps


_NC_CACHE = {}


def _get_nc(t_steps=T):
    if t_steps not in _NC_CACHE:
        _NC_CACHE[t_steps] = build_nc(t_steps)
    return _NC_CACHE[t_steps]


def kernel(x, h0, W_ih, W_hh, b_ih, b_hh, W_dense):
    x = np.asarray(x, dtype=np.float32)
    h0 = np.asarray(h0, dtype=np.float32)
    nc = _get_nc(T)
    in_maps = host_prep(x, h0, np.asarray(W_ih), np.asarray(W_hh),
                        np.asarray(b_ih), np.asarray(b_hh), np.asarray(W_dense))
    res = bass_utils.run_bass_kernel_spmd(nc, in_maps, core_ids=list(range(NCORES)))
    ys = np.empty((T, B, H), dtype=np.float32)
    for c in range(NCORES):
        ys[:, 16 * c:16 * c + BP, :] = res.results[c]["ys"]
    return ys



# revision 3
# speedup vs baseline: 1.3718x; 1.3718x over previous
"""Trainium2 Bass kernel for nn_DenseRNNBase (GRU with dense skip history), v2.

Sharding: data-parallel over batch B=128 across 8 cores. Each core covers 32
batch rows (16 primary + 16 wrap-around cover for 32-aligned PSUM regions)
and writes out only its 16 primary rows. Zero inter-core communication.

v2 restructure vs baseline:
- Two 3-bank PSUM areas: A holds regions (steps) 0-3 of a block, B holds 4-7.
  Banks per region: [r | z | n_rec].  The x-projection for r,z and ALL biases
  are matmul'd DIRECTLY into the areas (xproj opens r,z with start=True;
  K=1 ones-row matmuls add biases and open n_rec), killing the
  identity-inject and per-step b_hh_n matmuls of the baseline.
- The n-gate x part (i_n + b_ih_n) still goes via a PSUM scratch -> SBUF
  eviction since tanh needs i_n separate from the recurrent n term.
- Per step, the W_hh matmuls are emitted FIRST (r-bank chunks first so the
  sigmoid can start after 4 MMs), the GRU cell next, and all dense-lag pack
  streams AFTER the cell ops - so the packs execute on PE while the cell
  runs on ACT/DVE instead of blocking it.
- Dense-lag packing runs at the availability bound: 10 weight streams per
  8-step block cover all 21 lag terms (baseline: 11).
- (1-z)*n is fused as (z-1)*nn via scalar_tensor_tensor; hy = zh - that.
- ys DMA moved to the sync engine.
"""

import sys

for _p in ("/opt/trn_rl_repo",):
    if _p not in sys.path:
        sys.path.insert(0, _p)

import numpy as np
import ml_dtypes

import concourse.bass as bass
import concourse.bacc as bacc
import concourse.tile as tile
import concourse.mybir as mybir
from concourse import bass_utils

F32 = mybir.dt.float32
F32R = mybir.dt.float32r
BF16 = mybir.dt.bfloat16
AF = mybir.ActivationFunctionType
ALU = mybir.AluOpType

T, B, I, H, D = 256, 128, 512, 512, 8
G = 3 * H  # 1536
NCORES = 8
BL = 32   # local batch (double cover)
BP = 16   # primary batch rows written out
KI = I // 128
KH = H // 128

# Dense-lag weight streams, (lag d, first dest j1, cnt).
# MID_SCHED[j]: emitted mid-step j (after the cell ops, before the
# transposes): sources <= j-1, dests >= j+1 -- they fill the PE while the
# cell of step j runs, never blocking it.
# GAP_SCHED[j]: lag-1 pairs {j, j+1} whose source j-1 only exists after
# cell(j-1); they are emitted at the START of step j, bank-interleaved with
# the W_hh stream so sigma_r(j) is delayed by only the r-bank part.
# NOTE: matmul outputs must be naturally aligned in partition space --
# (base, span) of (0,64),(64,64),(0,128) or any 32-aligned span-32 window.
# Streams are chosen so every dest window satisfies that.
MID_SCHED = {
    1: [(6, 7, 1), (2, 3, 1)],
    2: [(5, 6, 2), (4, 5, 1)],
    3: [(3, 4, 2), (2, 4, 2)],
    4: [(3, 6, 2), (4, 6, 2)],
    5: [(2, 6, 2)],
}
GAP_SCHED = {2: [(1, 2, 2)], 4: [(1, 4, 2)], 6: [(1, 6, 2)]}


def build_nc(t_steps=T, reps=1):
    nc = bacc.Bacc("TRN2", target_bir_lowering=False, debug=False)

    xT_d = nc.dram_tensor("xT", (I, t_steps * BL), F32R, kind="ExternalInput")
    h0T_d = nc.dram_tensor("h0T", (H, BL), F32, kind="ExternalInput")
    h0n_d = nc.dram_tensor("h0n", (BL, H), F32, kind="ExternalInput")
    wih_d = nc.dram_tensor("wih", (I, G), F32R, kind="ExternalInput")
    whh_d = nc.dram_tensor("whh", (H, G), BF16, kind="ExternalInput")
    whh0_d = nc.dram_tensor("whh0", (H, G), BF16, kind="ExternalInput")
    wd_d = [nc.dram_tensor(f"wd{d}", (H, G), BF16, kind="ExternalInput")
            for d in range(1, 7)]
    biasn_d = nc.dram_tensor("biasn", (128, H), F32, kind="ExternalInput")
    ones128_d = nc.dram_tensor("ones128", (1, 128), BF16, kind="ExternalInput")
    brow_r_d = nc.dram_tensor("brow_r", (1, H), BF16, kind="ExternalInput")
    brow_z_d = nc.dram_tensor("brow_z", (1, H), BF16, kind="ExternalInput")
    brow_n_d = nc.dram_tensor("brow_n", (1, H), BF16, kind="ExternalInput")
    identf_d = nc.dram_tensor("identf", (128, BL), F32, kind="ExternalInput")
    ys_d = nc.dram_tensor("ys", (t_steps, BP, H), F32, kind="ExternalOutput")

    nblk = t_steps // 8

    with tile.TileContext(nc) as tc:
        with (
            tc.tile_pool(name="wpool", bufs=1) as wpool,
            tc.tile_pool(name="cpool", bufs=1) as cpool,
            tc.tile_pool(name="xt", bufs=2) as xtp,
            tc.tile_pool(name="sbn0", bufs=2) as sbn0p,
            tc.tile_pool(name="sbn1", bufs=2) as sbn1p,
            tc.tile_pool(name="hyp", bufs=2) as hyp,
            tc.tile_pool(name="ct", bufs=1) as ct,
            tc.tile_pool(name="paA", bufs=1, space="PSUM") as papA,
            tc.tile_pool(name="paB", bufs=1, space="PSUM") as papB,
            tc.tile_pool(name="px", bufs=1, space="PSUM") as pxp,
            tc.tile_pool(name="pt", bufs=1, space="PSUM") as ptp,
        ):
            # ---- resident weights ----
            wih = wpool.tile([128, KI, G], F32R, tag="wih")
            whh = wpool.tile([128, KH, G], BF16, tag="whh")
            whh0 = wpool.tile([128, KH, G], BF16, tag="whh0")
            wd = [wpool.tile([128, KH, G], BF16, tag=f"wd{d}", name=f"wd{d}")
                  for d in range(6)]
            for k in range(KI):
                nc.sync.dma_start(wih[:, k, :], wih_d[k * 128:(k + 1) * 128, :])
            for k in range(KH):
                nc.sync.dma_start(whh[:, k, :], whh_d[k * 128:(k + 1) * 128, :])
                nc.sync.dma_start(whh0[:, k, :], whh0_d[k * 128:(k + 1) * 128, :])
                for d in range(6):
                    nc.sync.dma_start(wd[d][:, k, :], wd_d[d][k * 128:(k + 1) * 128, :])

            biasn_s = cpool.tile([128, H], F32, tag="biasn")
            nc.sync.dma_start(biasn_s[:], biasn_d[:])
            ones_s = cpool.tile([1, 128], BF16, tag="ones128")
            nc.sync.dma_start(ones_s[:], ones128_d[:])
            brow_r = cpool.tile([1, H], BF16, tag="brow_r")
            nc.sync.dma_start(brow_r[:], brow_r_d[:])
            brow_z = cpool.tile([1, H], BF16, tag="brow_z")
            nc.sync.dma_start(brow_z[:], brow_z_d[:])
            brow_n = cpool.tile([1, H], BF16, tag="brow_n")
            nc.sync.dma_start(brow_n[:], brow_n_d[:])
            identf_s = cpool.tile([128, BL], F32, tag="identf")
            nc.sync.dma_start(identf_s[:], identf_d[:])

            # transposed hidden-state ring: slot s holds hyT of step t%8==s
            hist = cpool.tile([128, KH, 8, BL], BF16, tag="hist")
            h0T_s = cpool.tile([128, KH, BL], F32, tag="h0Ts")
            for k in range(KH):
                nc.sync.dma_start(h0T_s[:, k, :], h0T_d[k * 128:(k + 1) * 128, :])

            def xt_load(m, rep):
                xt_t = xtp.tile([128, KI, 8 * BL], F32R, tag="xt",
                                name=f"xt_{rep}_{m}")
                for k in range(KI):
                    nc.sync.dma_start(
                        xt_t[:, k, :],
                        xT_d[k * 128:(k + 1) * 128, m * 8 * BL:(m + 1) * 8 * BL])
                return xt_t

            def emit_xproj(xt_t, g, area, sbn):
                """x-projection + biases for one 4-step group into the
                per-bank tiles of `area` (list [r, z, n_rec]); n-part to PSUM
                scratch then SBUF with bias."""
                brows = (brow_r, brow_z)
                for k in range(KI):
                    for n in range(2):  # r, z banks: open with start on k==0
                        nc.tensor.matmul(
                            area[n][:, :],
                            xt_t[:, k, g * 128:(g + 1) * 128],
                            wih[:, k, n * 512:(n + 1) * 512],
                            start=(k == 0), stop=False, skip_group_check=True)
                # bias rows: r,z add; n_rec opens its bank
                for n in range(2):
                    nc.tensor.matmul(area[n][:, :], ones_s[:], brows[n][:],
                                     start=False, stop=False,
                                     skip_group_check=True)
                nc.tensor.matmul(area[2][:, :], ones_s[:], brow_n[:],
                                 start=True, stop=False, skip_group_check=True)
                px_t = pxp.tile([128, 512], F32, tag="px")
                for k in range(KI):
                    nc.tensor.matmul(
                        px_t[:],
                        xt_t[:, k, g * 128:(g + 1) * 128],
                        wih[:, k, 1024:1536],
                        start=(k == 0), stop=(k == KI - 1))
                nc.vector.tensor_add(sbn[:], px_t[:], biasn_s[:])

            for rep in range(reps):
                for k in range(KH):
                    nc.vector.tensor_copy(hist[:, k, 7, :], h0T_s[:, k, :])
                h_prev = hyp.tile([128, H], F32, tag="hy", name=f"hprev{rep}")
                nc.sync.dma_start(h_prev[96:128, :], h0n_d[:])
                po_prev = 96

                # one tile per PSUM bank: dep tracking (and the HW PE-write /
                # DVE-read collision rule) is bank-granular, so per-bank
                # tiles let packs write a bank as soon as ITS cell reader is
                # done instead of waiting for the whole region
                areaA = [papA.tile([128, 512], F32, tag=f"arA{n}",
                                   name=f"arA{n}") for n in range(3)]
                areaB = [papB.tile([128, 512], F32, tag=f"arB{n}",
                                   name=f"arB{n}") for n in range(3)]

                xt_cur = xt_load(0, rep)
                sbn_cur0 = sbn0p.tile([128, H], F32, tag="sbn0",
                                      name=f"sbn0_{rep}_0")
                emit_xproj(xt_cur, 0, areaA, sbn_cur0)
                xt_next = None

                for m in range(nblk):
                    sbn_cur1 = sbn1p.tile([128, H], F32, tag="sbn1",
                                          name=f"sbn1_{rep}_{m}")
                    sbn_next0 = None

                    def pack(d, j1, cnt):
                        """One W_dense[d] stream into regions j1..j1+cnt-1."""
                        ar = areaB if j1 >= 4 else areaA
                        pb = 32 * (j1 % 4)
                        s0 = j1 - 1 - d
                        tps = (0, 96) if pb == 96 else None
                        for k in range(KH):
                            for n in range(3):
                                nc.tensor.matmul(
                                    ar[n][pb:pb + 32 * cnt, :],
                                    hist[:, k, s0:s0 + cnt, :],
                                    wd[d - 1][:, k, n * 512:(n + 1) * 512],
                                    start=False, stop=False,
                                    tile_position=tps,
                                    skip_group_check=True)

                    for j in range(8):
                        t = 8 * m + j
                        po = 32 * (j % 4)
                        gi = j // 4
                        ar = areaB if j >= 4 else areaA
                        sbn_j = sbn_cur1 if j >= 4 else sbn_cur0
                        tp96 = (0, 96) if po == 96 else None

                        # ---- W_hh (+W_dense[0] for j>=1) ----
                        # r and n_rec banks in 256-col halves, interleaved, so
                        # the sigmoid/rn chain starts as early as possible;
                        # z bank (needed last) as full-width tail. A GAP_SCHED
                        # lag-1 pair stream is interleaved bank-by-bank.
                        gpk = GAP_SCHED.get(j, [])

                        def gap_bank(n):
                            for (d, j1, cnt) in gpk:
                                garea = areaB if j1 >= 4 else areaA
                                gpb = 32 * (j1 % 4)
                                gs0 = j1 - 1 - d
                                gtps = (0, 96) if gpb == 96 else None
                                for k in range(KH):
                                    nc.tensor.matmul(
                                        garea[n][gpb:gpb + 32 * cnt, :],
                                        hist[:, k, gs0:gs0 + cnt, :],
                                        wd[d - 1][:, k, n * 512:(n + 1) * 512],
                                        start=False, stop=False,
                                        tile_position=gtps,
                                        skip_group_check=True)

                        wsel = whh0 if j >= 1 else whh
                        hs = (j - 1) % 8

                        def whh_bank(n, c0, w):
                            for k in range(KH):
                                nc.tensor.matmul(
                                    ar[n][po:po + 32, c0:c0 + w],
                                    hist[:, k, hs, :],
                                    wsel[:, k, n * 512 + c0:n * 512 + c0 + w],
                                    start=False, stop=(k == KH - 1),
                                    tile_position=tp96,
                                    skip_group_check=True)

                        # ---- GRU cell tiles ----
                        s = slice(po, po + 32)
                        h_cur = ct.tile([128, H], F32, tag="hc")
                        nc.gpsimd.tensor_copy(
                            h_cur[s, :], h_prev[po_prev:po_prev + 32, :])
                        r = ct.tile([128, H], F32, tag="r")
                        z = ct.tile([128, H], F32, tag="z")
                        zh = ct.tile([128, H], F32, tag="zh")
                        rn = ct.tile([128, H], F32, tag="rn")
                        a_n = ct.tile([128, H], F32, tag="an")
                        nn = ct.tile([128, H], F32, tag="nn")
                        q = ct.tile([128, H], F32, tag="q")
                        hy = hyp.tile([128, H], F32, tag="hy")
                        ptt = ptp.tile([128, KH, BL], F32, tag="pt")
                        halves = [slice(0, 256), slice(256, 512)]

                        # whh bank groups interleaved with the cell ops that
                        # read them, so every PSUM read fires as early as the
                        # per-bank dep tracking allows
                        gap_bank(0)
                        for cs in halves:
                            whh_bank(0, cs.start, 256)      # r half
                            nc.scalar.activation(r[s, cs], ar[0][s, cs],
                                                 AF.Sigmoid)
                        gap_bank(2)
                        for cs in halves:
                            whh_bank(2, cs.start, 256)      # n_rec half
                            nc.vector.tensor_mul(rn[s, cs], r[s, cs],
                                                 ar[2][s, cs])
                            nc.vector.tensor_add(a_n[s, cs], rn[s, cs],
                                                 sbn_j[s, cs])
                        nc.scalar.activation(nn[s, halves[0]],
                                             a_n[s, halves[0]], AF.Tanh)
                        gap_bank(1)
                        for hi, cs in enumerate(halves):
                            whh_bank(1, cs.start, 256)      # z half
                            nc.scalar.activation(z[s, cs], ar[1][s, cs],
                                                 AF.Sigmoid)
                            if hi == 1:
                                nc.scalar.activation(nn[s, cs], a_n[s, cs],
                                                     AF.Tanh)
                            nc.gpsimd.tensor_mul(zh[s, cs], z[s, cs],
                                                 h_cur[s, cs])
                            # q = (z-1)*nn ; hy = zh - q = zh + (1-z)*nn
                            nc.vector.scalar_tensor_tensor(
                                q[s, cs], z[s, cs], 1.0, nn[s, cs],
                                ALU.subtract, ALU.mult)
                            nc.vector.tensor_sub(hy[s, cs], zh[s, cs], q[s, cs])

                        nc.sync.dma_start(ys_d[t], hy[po:po + BP, :])

                        # ---- mid-step PE fill: dense packs + xproj ----
                        # (before the transposes in the PE queue, so they run
                        # while the cell computes on ACT/DVE/Pool)
                        for (d, j1, cnt) in MID_SCHED.get(j, []):
                            pack(d, j1, cnt)
                        if j == 0:
                            # B regions 4..7 of THIS block: xproj g1 + biases
                            emit_xproj(xt_cur, 1, areaB, sbn_cur1)
                            if m + 1 < nblk:
                                xt_next = xt_load(m + 1, rep)
                        if j == 7 and m + 1 < nblk:
                            # A regions 0..3 of NEXT block: emitted during a
                            # B-reading cell, so the A-bank writes never wait
                            # on this block's cells
                            sbn_next0 = sbn0p.tile([128, H], F32, tag="sbn0",
                                                   name=f"sbn0_{rep}_{m + 1}")
                            emit_xproj(xt_next, 0, areaA, sbn_next0)

                        # ---- transposes feed the hist ring; whh(j+1) follows
                        # (copies alternate ACT/DVE so two run in parallel)
                        for k in range(KH):
                            nc.tensor.transpose(
                                ptt[:, k, :], hy[s, k * 128:(k + 1) * 128],
                                identf_s[s, :], tile_position=(po, 0))
                            if k % 2 == 0:
                                nc.scalar.copy(hist[:, k, j, :], ptt[:, k, :])
                            else:
                                nc.vector.tensor_copy(hist[:, k, j, :],
                                                      ptt[:, k, :])

                        h_prev = hy
                        po_prev = po
                    xt_cur = xt_next
                    sbn_cur0 = sbn_next0

    nc.compile()
    return nc


def round_f32r(a):
    """Round fp32 to the PE's float32r (tf32-like 10-bit mantissa)."""
    a = np.ascontiguousarray(a, dtype=np.float32)
    v = a.view(np.uint32).copy()
    v += 0x1000 + ((v >> 13) & 1)
    v &= 0xFFFFE000
    return v.view(np.float32)


def host_prep(x, h0, W_ih, W_hh, b_ih, b_hh, W_dense, t_steps=T):
    """Build per-core in_maps."""
    bf = ml_dtypes.bfloat16
    whh0 = (W_hh + W_dense[0]).astype(bf)
    whh_b = W_hh.astype(bf)
    wd_b = [W_dense[d].astype(bf) for d in range(1, 7)]
    brz = (b_ih + b_hh)[:2 * H]
    brow_r = brz[0:H].reshape(1, H).astype(bf)
    brow_z = brz[H:2 * H].reshape(1, H).astype(bf)
    brow_n = b_hh[2 * H:].reshape(1, H).astype(bf)
    biasn = np.broadcast_to(b_ih[2 * H:][None, :], (128, H)).astype(
        np.float32).copy()
    ones128 = np.ones((1, 128), bf)
    ident = np.tile(np.eye(BL, dtype=np.float32), (4, 1))
    wih_f = round_f32r(W_ih)

    in_maps = []
    for c in range(NCORES):
        idx = (16 * c + np.arange(BL)) % B
        xc = x[:t_steps, idx, :]
        xT = np.ascontiguousarray(xc.transpose(2, 0, 1).reshape(I, t_steps * BL))
        m = {
            "xT": round_f32r(xT),
            "h0T": np.ascontiguousarray(h0[idx].T).astype(np.float32),
            "h0n": h0[idx].astype(np.float32),
            "wih": wih_f, "whh": whh_b, "whh0": whh0,
            "biasn": biasn, "ones128": ones128,
            "brow_r": brow_r, "brow_z": brow_z, "brow_n": brow_n,
            "identf": ident,
        }
        for d in range(6):
            m[f"wd{d + 1}"] = wd_b[d]
        in_maps.append(m)
    return in_maps


_NC_CACHE = {}


def _get_nc(t_steps=T):
    if t_steps not in _NC_CACHE:
        _NC_CACHE[t_steps] = build_nc(t_steps)
    return _NC_CACHE[t_steps]


def kernel(x, h0, W_ih, W_hh, b_ih, b_hh, W_dense):
    x = np.asarray(x, dtype=np.float32)
    h0 = np.asarray(h0, dtype=np.float32)
    nc = _get_nc(T)
    in_maps = host_prep(x, h0, np.asarray(W_ih), np.asarray(W_hh),
                        np.asarray(b_ih), np.asarray(b_hh), np.asarray(W_dense))
    res = bass_utils.run_bass_kernel_spmd(nc, in_maps, core_ids=list(range(NCORES)))
    ys = np.empty((T, B, H), dtype=np.float32)
    for c in range(NCORES):
        ys[:, 16 * c:16 * c + BP, :] = res.results[c]["ys"]
    return ys
